# revision 2
# baseline (speedup 1.0000x reference)
"""Trainium2 Bass kernel for nn_ConvSpatialPropagationNet_71949292143069.

Reference semantics (B=8, C=8, H=352, W=1216, STEP=24):

    gate_wb  = eight_way_pad(guidance)            # [B,8,H+2,W+2], per-channel shift
    A        = (sum_c w_c * g_c) / (sum_c w_c * |g_c|)   # per canvas pixel
    gate_sum = A[:, 1:-1, 1:-1]
    d_0      = raw = blur_depth
    step:  nws = (A * pad(d))[:, 1:-1, 1:-1]      # SAME canvas window as gate_sum
           d   = (1 - gate_sum) * raw + nws
           d   = (1 - mask) * d + mask * raw

Key algebraic identity: `gate_wb * dp` and `gate_sum` are evaluated at the
same canvas position and cropped with the same [1:-1,1:-1] window, so
nws == gate_sum * d elementwise.  The update is therefore the pointwise
recurrence d' = (1-A)*raw + A*d with d_0 = raw, whose exact fixed point is
d = raw for ANY A, and the mask step preserves it.  The module's exact
output is blur_depth (verified: reference differs from blur_depth by at
most ~4e-6 absolute / ~5e-6 relative — pure fp32 rounding noise, since
|A| <= 1 when sum_w > 0).

The optimal correct kernel is therefore a device-side copy of blur_depth
to the output, sharded batch-parallel over the 8 NeuronCores (1 batch per
core, one ~1.7MB DRAM->DRAM DMA each) — the memory-roofline floor of
read-output-bytes + write-output-bytes.
"""

import numpy as np

B, H, W = 8, 352, 1216
N = H * W

_CACHE = {}


def _build_program():
    import concourse.bass as bass
    import concourse.mybir as mybir

    nc = bass.Bass()
    x = nc.declare_dram_parameter("x", [N], mybir.dt.float32, isOutput=False)
    y = nc.declare_dram_parameter("y", [N], mybir.dt.float32, isOutput=True)
    with nc.Block() as block, nc.semaphore("dma_sem") as dma_sem:

        @block.sync
        def _(sync):
            sync.dma_start(out=y[:], in_=x[:]).then_inc(dma_sem, 16)
            sync.wait_ge(dma_sem, 16)

    return nc


def _run(blur: np.ndarray, trace: bool = False, **spmd_kwargs):
    from concourse.bass_utils import run_bass_kernel_spmd

    if "nc" not in _CACHE:
        _CACHE["nc"] = _build_program()
    nc = _CACHE["nc"]

    in_maps = [{"x": blur[i, 0].reshape(N)} for i in range(B)]
    bkr = run_bass_kernel_spmd(nc, in_maps, list(range(B)), trace=trace, **spmd_kwargs)
    out = np.stack([np.asarray(bkr.results[i]["y"]) for i in range(B)], axis=0)
    return out.reshape(B, 1, H, W).astype(np.float32, copy=False), bkr


def kernel(**inputs: np.ndarray) -> np.ndarray:
    blur = np.ascontiguousarray(inputs["blur_depth"], dtype=np.float32)
    assert blur.shape == (B, 1, H, W), blur.shape
    return _run(blur)[0]


if __name__ == "__main__":
    rng = np.random.default_rng(0)
    ins = {
        "guidance": rng.standard_normal((B, 8, H, W), dtype=np.float32),
        "blur_depth": rng.random((B, 1, H, W), dtype=np.float32),
        "sparse_depth": rng.random((B, 1, H, W), dtype=np.float32),
        "sum_w": (rng.standard_normal(8) * 0.1 + 1.0).astype(np.float32),
    }
    out = kernel(**ins)
    print("kernel ran; max abs diff vs blur:", np.abs(out - ins["blur_depth"]).max())


# revision 3
# speedup vs baseline: 1.2248x; 1.2248x over previous
"""Trainium2 Bass kernel for nn_ConvSpatialPropagationNet_71949292143069.

Reference semantics (B=8, C=8, H=352, W=1216, STEP=24):

    gate_wb  = eight_way_pad(guidance)            # [B,8,H+2,W+2], per-channel shift
    A        = (sum_c w_c * g_c) / (sum_c w_c * |g_c|)   # per canvas pixel
    gate_sum = A[:, 1:-1, 1:-1]
    d_0      = raw = blur_depth
    step:  nws = (A * pad(d))[:, 1:-1, 1:-1]      # SAME canvas window as gate_sum
           d   = (1 - gate_sum) * raw + nws
           d   = (1 - mask) * d + mask * raw

Key algebraic identity: `gate_wb * dp` and `gate_sum` are evaluated at the
same canvas position and cropped with the same [1:-1,1:-1] window, so
nws == gate_sum * d elementwise.  The update is therefore the pointwise
recurrence d' = (1-A)*raw + A*d with d_0 = raw, whose exact fixed point is
d = raw for ANY A, and the mask step preserves it.  The module's exact
output is blur_depth (verified: reference differs from blur_depth by at
most ~4e-6 absolute / ~5e-6 relative — pure fp32 rounding noise, since
|A| <= 1 when sum_w > 0).

The optimal correct kernel is therefore a device-side copy of blur_depth
to the output, sharded batch-parallel over the 8 NeuronCores (1 batch per
core, one ~1.7MB DRAM->DRAM DMA each) — the memory-roofline floor of
read-output-bytes + write-output-bytes.
"""

import numpy as np

B, H, W = 8, 352, 1216
N = H * W

_CACHE = {}


def _build_program():
    import concourse.bass as bass
    import concourse.mybir as mybir

    nc = bass.Bass()
    x = nc.declare_dram_parameter("x", [N], mybir.dt.float32, isOutput=False)
    y = nc.declare_dram_parameter("y", [N], mybir.dt.float32, isOutput=True)
    with nc.Block() as block, nc.semaphore("dma_sem") as dma_sem:

        @block.sync
        def _(sync):
            sync.dma_start(out=y[:], in_=x[:]).then_inc(dma_sem, 16)
            sync.wait_ge(dma_sem, 16)

    return nc


def _run(blur: np.ndarray, trace: bool = False, **spmd_kwargs):
    from concourse.bass_utils import run_bass_kernel_spmd

    if "nc" not in _CACHE:
        _CACHE["nc"] = _build_program()
    nc = _CACHE["nc"]

    in_maps = [{"x": blur[i, 0].reshape(N)} for i in range(B)]
    bkr = run_bass_kernel_spmd(nc, in_maps, list(range(B)), trace=trace, **spmd_kwargs)
    out = np.stack([np.asarray(bkr.results[i]["y"]) for i in range(B)], axis=0)
    return out.reshape(B, 1, H, W).astype(np.float32, copy=False), bkr


def kernel(**inputs: np.ndarray) -> np.ndarray:
    blur = np.ascontiguousarray(inputs["blur_depth"], dtype=np.float32)
    assert blur.shape == (B, 1, H, W), blur.shape
    # One rebuild+retry: the axon-proxied NRT occasionally throws a transient
    # NRT_EXEC_UNIT_UNRECOVERABLE on execute; a fresh attempt succeeds.
    try:
        return _run(blur)[0]
    except Exception:
        _CACHE.clear()
        return _run(blur)[0]


if __name__ == "__main__":
    rng = np.random.default_rng(0)
    ins = {
        "guidance": rng.standard_normal((B, 8, H, W), dtype=np.float32),
        "blur_depth": rng.random((B, 1, H, W), dtype=np.float32),
        "sparse_depth": rng.random((B, 1, H, W), dtype=np.float32),
        "sum_w": (rng.standard_normal(8) * 0.1 + 1.0).astype(np.float32),
    }
    out = kernel(**ins)
    print("kernel ran; max abs diff vs blur:", np.abs(out - ins["blur_depth"]).max())


# revision 4
# speedup vs baseline: 1.2961x; 1.0582x over previous
"""Trainium2 Bass kernel for nn_ConvSpatialPropagationNet_71949292143069.

Reference semantics (B=8, C=8, H=352, W=1216, STEP=24):

    gate_wb  = eight_way_pad(guidance)            # [B,8,H+2,W+2], per-channel shift
    A        = (sum_c w_c * g_c) / (sum_c w_c * |g_c|)   # per canvas pixel
    gate_sum = A[:, 1:-1, 1:-1]
    d_0      = raw = blur_depth
    step:  nws = (A * pad(d))[:, 1:-1, 1:-1]      # SAME canvas window as gate_sum
           d   = (1 - gate_sum) * raw + nws
           d   = (1 - mask) * d + mask * raw

Key algebraic identity: `gate_wb * dp` and `gate_sum` are evaluated at the
same canvas position and cropped with the same [1:-1,1:-1] window, so
nws == gate_sum * d elementwise.  The update is therefore the pointwise
recurrence d' = (1-A)*raw + A*d with d_0 = raw, whose exact fixed point is
d = raw for ANY A, and the mask step preserves it.  The module's exact
output is blur_depth (verified: reference differs from blur_depth by at
most ~4e-6 absolute / ~5e-6 relative — pure fp32 rounding noise, since
|A| <= 1 when sum_w > 0).

The optimal correct kernel is therefore a device-side copy of blur_depth
to the output, sharded batch-parallel over the 8 NeuronCores (1 batch per
core, one ~1.7MB DRAM->DRAM DMA each) — the memory-roofline floor of
read-output-bytes + write-output-bytes.
"""

import numpy as np

B, H, W = 8, 352, 1216
N = H * W

_CACHE = {}


def _build_program():
    import concourse.bass as bass
    import concourse.mybir as mybir

    nc = bass.Bass()
    x = nc.declare_dram_parameter("x", [N], mybir.dt.float32, isOutput=False)
    y = nc.declare_dram_parameter("y", [N], mybir.dt.float32, isOutput=True)
    # Raw single-engine program (no Block): skips the block-exit all-engine
    # barrier, whose post-DMA semaphore chatter counts against exec time
    # (~1.1us, measured via interleaved A/B). The wait_ge keeps the output
    # write ordered before kernel end.
    with nc.semaphore("dma_sem") as dma_sem:
        nc.sync.dma_start(out=y[:], in_=x[:]).then_inc(dma_sem, 16)
        nc.sync.wait_ge(dma_sem, 16)

    return nc


def _run(blur: np.ndarray, trace: bool = False, **spmd_kwargs):
    from concourse.bass_utils import run_bass_kernel_spmd

    if "nc" not in _CACHE:
        _CACHE["nc"] = _build_program()
    nc = _CACHE["nc"]

    in_maps = [{"x": blur[i, 0].reshape(N)} for i in range(B)]
    bkr = run_bass_kernel_spmd(nc, in_maps, list(range(B)), trace=trace, **spmd_kwargs)
    out = np.stack([np.asarray(bkr.results[i]["y"]) for i in range(B)], axis=0)
    return out.reshape(B, 1, H, W).astype(np.float32, copy=False), bkr


def kernel(**inputs: np.ndarray) -> np.ndarray:
    blur = np.ascontiguousarray(inputs["blur_depth"], dtype=np.float32)
    assert blur.shape == (B, 1, H, W), blur.shape
    # One rebuild+retry: the axon-proxied NRT occasionally throws a transient
    # NRT_EXEC_UNIT_UNRECOVERABLE on execute; a fresh attempt succeeds.
    try:
        return _run(blur)[0]
    except Exception:
        _CACHE.clear()
        return _run(blur)[0]


if __name__ == "__main__":
    rng = np.random.default_rng(0)
    ins = {
        "guidance": rng.standard_normal((B, 8, H, W), dtype=np.float32),
        "blur_depth": rng.random((B, 1, H, W), dtype=np.float32),
        "sparse_depth": rng.random((B, 1, H, W), dtype=np.float32),
        "sum_w": (rng.standard_normal(8) * 0.1 + 1.0).astype(np.float32),
    }
    out = kernel(**ins)
    print("kernel ran; max abs diff vs blur:", np.abs(out - ins["blur_depth"]).max())
